# revision 6
# baseline (speedup 1.0000x reference)
"""Trainium2 Bass kernel for nn_DepthwiseCrossViTMAE (criss-cross multihead self-attention).

Reference computation per token t (B*L = 4096 tokens, hidden 2048 = C*K with C=32, K=64):
  qkv[c, :] = x[t, c*64:(c+1)*64] @ Wqkv[c] + bqkv[c]          (per-channel linear)
  q, k, v = split(qkv)                                          each (C, K)
  for each d in [0, 64):  S_d = outer(q[:, d], k[:, d]) / 8     (C x C)
                          A_d = softmax_rows(S_d)
                          ctx[d, m] = sum_c A_d[c, m] * v[c, d]
  out[t, c*64:(c+1)*64] = ctx.T[c] @ Wout + bout

Sharding: data-parallel over the 4096 tokens, 512 tokens per core on 8 cores.

Per-core layout: SBUF partitions p = (t2, d) with t2 in {0,1}, d in [0,64);
token t = t2*256 + tau.  q/k/v live as [p, (tau, c)].  The scores tensor is
built per chunk of Tc tau's as [p, (tau, c, e)] by a broadcast tensor_tensor
outer product on DVE, exp on ACT, row-sums (over e) and weighted column-sums
(over c) as segmented tensor_reduce on DVE.  PE does the QKV projections
(contracting k=64 per channel), the x transposes, and the output projection.
"""

import sys

sys.path.insert(0, "/opt/trn_rl_repo")

import numpy as np
from contextlib import ExitStack

import concourse.bass as bass
import concourse.bacc as bacc
import concourse.mybir as mybir
import concourse.tile as tile
from concourse.masks import make_identity

F32 = mybir.dt.float32
AF = mybir.ActivationFunctionType
ALU = mybir.AluOpType
AX = mybir.AxisListType

C = 32          # channels
K = 64          # per-channel width (also Wout dim)
F = C * K       # hidden = 2048
N_CORES = 8


def build_nc(T=512, Tc=4, flush=8):
    """Build the single-core Bass module for T tokens.

    Tc: tau-chunk size for the attention loop (free size per DVE op = Tc*1024).
    flush: chunks per output-DMA flush group.
    """
    T2 = 2
    TH = T // T2            # tokens per half (tau range)
    NFC = F // 128          # 16 feature chunks of x
    NTT = (T + 127) // 128  # token tiles of x
    NCH = TH // Tc          # attention chunks
    assert TH % Tc == 0 and NCH % flush == 0

    nc = bacc.Bacc()
    x_d = nc.dram_tensor("x", [T, F], F32, kind="ExternalInput")
    wq_d = nc.dram_tensor("wq", [F, K], F32, kind="ExternalInput")
    wk_d = nc.dram_tensor("wk", [F, K], F32, kind="ExternalInput")
    wv_d = nc.dram_tensor("wv", [F, K], F32, kind="ExternalInput")
    bq_d = nc.dram_tensor("bq", [128, C], F32, kind="ExternalInput")
    bk_d = nc.dram_tensor("bk", [128, C], F32, kind="ExternalInput")
    bv_d = nc.dram_tensor("bv", [128, C], F32, kind="ExternalInput")
    wo_d = nc.dram_tensor("wout", [128, K], F32, kind="ExternalInput")
    bo_d = nc.dram_tensor("bout", [128, K], F32, kind="ExternalInput")
    out_d = nc.dram_tensor("out", [T, F], F32, kind="ExternalOutput")

    with ExitStack() as octx:
        tc = octx.enter_context(tile.TileContext(nc))
        const_pool = octx.enter_context(tc.tile_pool(name="const", bufs=1))
        qkv_pool = octx.enter_context(tc.tile_pool(name="qkv", bufs=1))

        ident = const_pool.tile([128, 128], F32)
        make_identity(nc, ident[:])

        bq_sb = const_pool.tile([128, C], F32)
        bk_sb = const_pool.tile([128, C], F32)
        bv_sb = const_pool.tile([128, C], F32)
        wo_sb = const_pool.tile([128, K], F32)
        bo_sb = const_pool.tile([128, K], F32)
        nc.sync.dma_start(bq_sb[:], bq_d[:])
        nc.sync.dma_start(bk_sb[:], bk_d[:])
        nc.sync.dma_start(bv_sb[:], bv_d[:])
        nc.sync.dma_start(wo_sb[:], wo_d[:])
        nc.sync.dma_start(bo_sb[:], bo_d[:])

        # q is pre-scaled by 1/sqrt(K) on the host (weights and bias).
        qs_sb = qkv_pool.tile([128, TH, C], F32)
        k_sb = qkv_pool.tile([128, TH, C], F32)
        v_sb = qkv_pool.tile([128, TH, C], F32)

        # ---------------- phase 1: load x, transpose, QKV projections ----
        with (
            tc.tile_pool(name="xload", bufs=2) as xpool,
            tc.tile_pool(name="xt", bufs=1) as xtpool,
            tc.tile_pool(name="wgt", bufs=1) as wpool,
            tc.tile_pool(name="ps_qkv", bufs=6, space="PSUM") as ps1,
            tc.tile_pool(name="ps_tr", bufs=2, space="PSUM") as pst,
        ):
            wq_sb = wpool.tile([128, NFC, K], F32)
            wk_sb = wpool.tile([128, NFC, K], F32)
            wv_sb = wpool.tile([128, NFC, K], F32)
            nc.sync.dma_start(wq_sb[:], wq_d[:].rearrange("(fc p) d -> p fc d", p=128))
            nc.sync.dma_start(wk_sb[:], wk_d[:].rearrange("(fc p) d -> p fc d", p=128))
            nc.sync.dma_start(wv_sb[:], wv_d[:].rearrange("(fc p) d -> p fc d", p=128))

            # xT[feat, token] per 128-feature chunk, via PE transpose
            xt_sb = xtpool.tile([128, NFC, T], F32)
            for tt in range(NTT):
                trows = min(128, T - tt * 128)
                x_sb = xpool.tile([128, F], F32)
                for fc in range(NFC):
                    nc.sync.dma_start(
                        x_sb[:trows, fc * 128 : (fc + 1) * 128],
                        x_d[tt * 128 : tt * 128 + trows, fc * 128 : (fc + 1) * 128],
                    )
                for fc in range(NFC):
                    ps_t = pst.tile([128, 128], F32)
                    nc.tensor.transpose(
                        ps_t[:, :trows],
                        x_sb[:trows, fc * 128 : (fc + 1) * 128],
                        ident[:trows, :trows],
                    )
                    nc.scalar.copy(
                        xt_sb[:, fc, tt * 128 : tt * 128 + trows], ps_t[:, :trows]
                    )

            # per-channel QKV projections
            for c in range(C):
                fc, h = divmod(c, 2)
                hp = slice(64 * h, 64 * h + 64)
                for w_sb, b_sb, dst in (
                    (wq_sb, bq_sb, qs_sb),
                    (wk_sb, bk_sb, k_sb),
                    (wv_sb, bv_sb, v_sb),
                ):
                    ps = ps1.tile([128, TH], F32)
                    for t2 in range(T2):
                        nc.tensor.matmul(
                            ps[64 * t2 : 64 * t2 + 64, :],
                            w_sb[hp, fc, :],
                            xt_sb[hp, fc, t2 * TH : (t2 + 1) * TH],
                            start=True,
                            stop=True,
                        )
                    nc.scalar.activation(
                        dst[:, :, c], ps[:], AF.Identity, bias=b_sb[:, c : c + 1]
                    )

        # ---------------- phase 2: criss-cross attention + out-proj ------
        with (
            tc.tile_pool(name="s", bufs=2) as s_pool,
            tc.tile_pool(name="e", bufs=2) as e_pool,
            tc.tile_pool(name="zsm", bufs=3) as z_pool,
            tc.tile_pool(name="ctx", bufs=2) as ctx_pool,
            tc.tile_pool(name="stage", bufs=2) as stage_pool,
            tc.tile_pool(name="ps_o", bufs=4, space="PSUM") as ps2,
        ):
            stage = [None, None]
            for ch in range(NCH):
                g = ch % flush
                if g == 0:
                    stage = [
                        stage_pool.tile([128, flush, K], F32, tag="st0", name="st0"),
                        stage_pool.tile([128, flush, K], F32, tag="st1", name="st1"),
                    ]
                tsl = slice(ch * Tc, (ch + 1) * Tc)
                qs4 = qs_sb[:, tsl, :].unsqueeze(3).broadcast_to([128, Tc, C, C])
                k4 = k_sb[:, tsl, :].unsqueeze(2).broadcast_to([128, Tc, C, C])
                s_t = s_pool.tile([128, Tc, C, C], F32)
                nc.vector.tensor_tensor(s_t[:], qs4, k4, ALU.mult)

                e_t = e_pool.tile([128, Tc, C, C], F32)
                nc.scalar.activation(e_t[:], s_t[:], AF.Exp)

                z_t = z_pool.tile([128, Tc, C], F32, tag="z")
                nc.vector.tensor_reduce(z_t[:], e_t[:], AX.X, ALU.add)
                zi_t = z_pool.tile([128, Tc, C], F32, tag="zi")
                nc.vector.reciprocal(zi_t[:], z_t[:])
                w_t = z_pool.tile([128, Tc, C], F32, tag="w")
                nc.vector.tensor_tensor(w_t[:], v_sb[:, tsl, :], zi_t[:], ALU.mult)

                # P = E * W (broadcast over e), written into s_t (S is dead)
                w4 = w_t[:].unsqueeze(3).broadcast_to([128, Tc, C, C])
                nc.vector.tensor_tensor(s_t[:], e_t[:], w4, ALU.mult)

                # ctx[p, tau, m] = sum_c P[p, tau, c, m]
                ctx_t = ctx_pool.tile([128, Tc, C], F32)
                nc.vector.tensor_reduce(
                    ctx_t[:], s_t[:].transpose([0, 1, 3, 2]), AX.X, ALU.add
                )

                # out-proj: out[tau, m, o] = sum_d ctx[(t2,d),(tau,m)] * Wout[d, o]
                for t2 in range(T2):
                    dp = slice(64 * t2, 64 * t2 + 64)
                    po = ps2.tile([128, K], F32)
                    nc.tensor.matmul(
                        po[:],
                        ctx_t[dp, :, :].rearrange("p t c -> p (t c)"),
                        wo_sb[dp, :],
                        start=True,
                        stop=True,
                    )
                    nc.vector.tensor_tensor(
                        stage[t2][:, g, :], po[:], bo_sb[:], ALU.add
                    )

                if g == flush - 1:
                    chb = ch // flush
                    ov = out_d[:].rearrange(
                        "(t2 chb chs tau) (m o) -> t2 chb tau m chs o",
                        t2=T2,
                        chb=NCH // flush,
                        chs=flush,
                        tau=Tc,
                        m=C,
                    )
                    for t2 in range(T2):
                        nc.sync.dma_start(ov[t2, chb], stage[t2][:])

    nc.compile()
    return nc


def _host_prep(x, Wqkv, bqkv, Wout, bout):
    x = np.ascontiguousarray(np.asarray(x, dtype=np.float32)).reshape(-1, F)
    Wqkv = np.asarray(Wqkv, dtype=np.float32)
    bqkv = np.asarray(bqkv, dtype=np.float32)
    Wout = np.asarray(Wout, dtype=np.float32)
    bout = np.asarray(bout, dtype=np.float32)
    scale = 1.0 / np.sqrt(K)

    common = {
        "wq": np.ascontiguousarray((Wqkv[:, :, :K] * scale).reshape(F, K)),
        "wk": np.ascontiguousarray(Wqkv[:, :, K : 2 * K].reshape(F, K)),
        "wv": np.ascontiguousarray(Wqkv[:, :, 2 * K :].reshape(F, K)),
        "bq": np.ascontiguousarray(np.tile((bqkv[:, :K] * scale).T, (2, 1))),
        "bk": np.ascontiguousarray(np.tile(bqkv[:, K : 2 * K].T, (2, 1))),
        "bv": np.ascontiguousarray(np.tile(bqkv[:, 2 * K :].T, (2, 1))),
        "wout": np.ascontiguousarray(np.tile(Wout, (2, 1))),
        "bout": np.ascontiguousarray(np.tile(bout[None, :], (128, 1))),
    }
    return x, common


_NC_CACHE = {}


def _get_nc(T):
    if T not in _NC_CACHE:
        _NC_CACHE[T] = build_nc(T=T)
    return _NC_CACHE[T]


def kernel(x, Wqkv, bqkv, Wout, bout, _trace=False):
    from concourse.bass_utils import run_bass_kernel_spmd

    xs, common = _host_prep(x, Wqkv, bqkv, Wout, bout)
    n_tok = xs.shape[0]
    tpc = n_tok // N_CORES
    in_maps = [
        {**common, "x": np.ascontiguousarray(xs[i * tpc : (i + 1) * tpc])}
        for i in range(N_CORES)
    ]
    nc = _get_nc(tpc)
    res = run_bass_kernel_spmd(nc, in_maps, list(range(N_CORES)), trace=_trace)
    out = np.concatenate([res.results[i]["out"] for i in range(N_CORES)], axis=0)
    out = out.reshape(np.asarray(x).shape)
    if _trace:
        kernel.last_results = res
    return out


# revision 9
# speedup vs baseline: 1.0323x; 1.0323x over previous
"""Trainium2 Bass kernel for nn_DepthwiseCrossViTMAE (criss-cross multihead self-attention).

Reference computation per token t (B*L = 4096 tokens, hidden 2048 = C*K with C=32, K=64):
  qkv[c, :] = x[t, c*64:(c+1)*64] @ Wqkv[c] + bqkv[c]          (per-channel linear)
  q, k, v = split(qkv)                                          each (C, K)
  for each d in [0, 64):  S_d = outer(q[:, d], k[:, d]) / 8     (C x C)
                          A_d = softmax_rows(S_d)
                          ctx[d, m] = sum_c A_d[c, m] * v[c, d]
  out[t, c*64:(c+1)*64] = ctx.T[c] @ Wout + bout

Sharding: data-parallel over the 4096 tokens, 512 tokens per core on 8 cores.

Per-core layout: SBUF partitions p = (t2, d) with t2 in {0,1}, d in [0,64);
token t = t2*256 + tau.  q/k/v live as [p, (tau, c)].  The scores tensor is
built per chunk of Tc tau's as [p, (tau, c, e)] by a broadcast tensor_tensor
outer product on DVE, exp on ACT, row-sums (over e) and weighted column-sums
(over c) as segmented tensor_reduce on DVE.  PE does the QKV projections
(contracting k=64 per channel), the x transposes, and the output projection.
"""

import sys

sys.path.insert(0, "/opt/trn_rl_repo")

import numpy as np
from contextlib import ExitStack

import concourse.bass as bass
import concourse.bacc as bacc
import concourse.mybir as mybir
import concourse.tile as tile
from concourse.masks import make_identity

F32 = mybir.dt.float32
AF = mybir.ActivationFunctionType
ALU = mybir.AluOpType
AX = mybir.AxisListType

C = 32          # channels
K = 64          # per-channel width (also Wout dim)
F = C * K       # hidden = 2048
N_CORES = 8


def build_nc(T=512, Tc=4, flush=8, gp_num=3, gp_den=8):
    """Build the single-core Bass module for T tokens.

    Tc: tau-chunk size for the attention loop (free size per DVE op = Tc*1024).
    flush: chunks per output-DMA flush group.
    gp_num/gp_den: chunks with ch % gp_den < gp_num run their elementwise
    score/P passes on GPSIMD instead of DVE (engine load balancing).
    """
    T2 = 2
    TH = T // T2            # tokens per half (tau range)
    NFC = F // 128          # 16 feature chunks of x
    NTT = (T + 127) // 128  # token tiles of x
    NCH = TH // Tc          # attention chunks
    assert TH % Tc == 0 and NCH % flush == 0

    nc = bacc.Bacc()
    x_d = nc.dram_tensor("x", [T, F], F32, kind="ExternalInput")
    wq_d = nc.dram_tensor("wq", [F, K], F32, kind="ExternalInput")
    wk_d = nc.dram_tensor("wk", [F, K], F32, kind="ExternalInput")
    wv_d = nc.dram_tensor("wv", [F, K], F32, kind="ExternalInput")
    bq_d = nc.dram_tensor("bq", [128, C], F32, kind="ExternalInput")
    bk_d = nc.dram_tensor("bk", [128, C], F32, kind="ExternalInput")
    bv_d = nc.dram_tensor("bv", [128, C], F32, kind="ExternalInput")
    wo_d = nc.dram_tensor("wout", [128, K], F32, kind="ExternalInput")
    bo_d = nc.dram_tensor("bout", [128, K], F32, kind="ExternalInput")
    out_d = nc.dram_tensor("out", [T, F], F32, kind="ExternalOutput")

    with ExitStack() as octx:
        tc = octx.enter_context(tile.TileContext(nc))
        const_pool = octx.enter_context(tc.tile_pool(name="const", bufs=1))
        qkv_pool = octx.enter_context(tc.tile_pool(name="qkv", bufs=1))

        ident = const_pool.tile([128, 128], F32)
        make_identity(nc, ident[:])
        ones_row = const_pool.tile([1, 128], F32)
        nc.gpsimd.memset(ones_row[:], 1.0)

        bq_sb = const_pool.tile([128, C], F32)
        bk_sb = const_pool.tile([128, C], F32)
        bv_sb = const_pool.tile([128, C], F32)
        wo_sb = const_pool.tile([128, K], F32)
        bo_sb = const_pool.tile([128, K], F32)
        nc.sync.dma_start(bq_sb[:], bq_d[:])
        nc.sync.dma_start(bk_sb[:], bk_d[:])
        nc.sync.dma_start(bv_sb[:], bv_d[:])
        nc.sync.dma_start(wo_sb[:], wo_d[:])
        nc.sync.dma_start(bo_sb[:], bo_d[:])

        # q is pre-scaled by 1/sqrt(K) on the host (weights and bias).
        qs_sb = qkv_pool.tile([128, TH, C], F32)
        k_sb = qkv_pool.tile([128, TH, C], F32)
        v_sb = qkv_pool.tile([128, TH, C], F32)

        # ---------------- phase 1: load x, transpose, QKV projections ----
        with (
            tc.tile_pool(name="xload", bufs=2) as xpool,
            tc.tile_pool(name="xt", bufs=1) as xtpool,
            tc.tile_pool(name="wgt", bufs=1) as wpool,
            tc.tile_pool(name="ps_qkv", bufs=6, space="PSUM") as ps1,
            tc.tile_pool(name="ps_tr", bufs=2, space="PSUM") as pst,
        ):
            wq_sb = wpool.tile([128, NFC, K], F32)
            wk_sb = wpool.tile([128, NFC, K], F32)
            wv_sb = wpool.tile([128, NFC, K], F32)
            nc.sync.dma_start(wq_sb[:], wq_d[:].rearrange("(fc p) d -> p fc d", p=128))
            nc.sync.dma_start(wk_sb[:], wk_d[:].rearrange("(fc p) d -> p fc d", p=128))
            nc.sync.dma_start(wv_sb[:], wv_d[:].rearrange("(fc p) d -> p fc d", p=128))

            # xT[feat, token] per 128-feature chunk, via PE transpose
            xt_sb = xtpool.tile([128, NFC, T], F32)
            for tt in range(NTT):
                trows = min(128, T - tt * 128)
                x_sb = xpool.tile([128, F], F32)
                for fc in range(NFC):
                    nc.sync.dma_start(
                        x_sb[:trows, fc * 128 : (fc + 1) * 128],
                        x_d[tt * 128 : tt * 128 + trows, fc * 128 : (fc + 1) * 128],
                    )
                for fc in range(NFC):
                    ps_t = pst.tile([128, 128], F32)
                    nc.tensor.transpose(
                        ps_t[:, :trows],
                        x_sb[:trows, fc * 128 : (fc + 1) * 128],
                        ident[:trows, :trows],
                    )
                    nc.scalar.copy(
                        xt_sb[:, fc, tt * 128 : tt * 128 + trows], ps_t[:, :trows]
                    )

            # per-channel QKV projections
            for c in range(C):
                fc, h = divmod(c, 2)
                hp = slice(64 * h, 64 * h + 64)
                for w_sb, b_sb, dst in (
                    (wq_sb, bq_sb, qs_sb),
                    (wk_sb, bk_sb, k_sb),
                    (wv_sb, bv_sb, v_sb),
                ):
                    ps = ps1.tile([128, TH], F32)
                    for t2 in range(T2):
                        nc.tensor.matmul(
                            ps[64 * t2 : 64 * t2 + 64, :],
                            w_sb[hp, fc, :],
                            xt_sb[hp, fc, t2 * TH : (t2 + 1) * TH],
                            start=True,
                            stop=True,
                        )
                    nc.scalar.activation(
                        dst[:, :, c], ps[:], AF.Identity, bias=b_sb[:, c : c + 1]
                    )

        # ---------------- phase 2: criss-cross attention + out-proj ------
        with (
            tc.tile_pool(name="s", bufs=2) as s_pool,
            tc.tile_pool(name="e", bufs=2) as e_pool,
            tc.tile_pool(name="zsm", bufs=3) as z_pool,
            tc.tile_pool(name="ctx", bufs=2) as ctx_pool,
            tc.tile_pool(name="stage", bufs=2) as stage_pool,
            tc.tile_pool(name="ps_o", bufs=4, space="PSUM") as ps2,
        ):
            stage = [None, None]
            for ch in range(NCH):
                g = ch % flush
                if g == 0:
                    stage = [
                        stage_pool.tile([128, flush, K], F32, tag="st0", name="st0"),
                        stage_pool.tile([128, flush, K], F32, tag="st1", name="st1"),
                    ]
                tsl = slice(ch * Tc, (ch + 1) * Tc)
                ew = nc.gpsimd if (ch % gp_den) < gp_num else nc.vector
                qs4 = qs_sb[:, tsl, :].unsqueeze(3).broadcast_to([128, Tc, C, C])
                k4 = k_sb[:, tsl, :].unsqueeze(2).broadcast_to([128, Tc, C, C])
                s_t = s_pool.tile([128, Tc, C, C], F32)
                ew.tensor_tensor(s_t[:], qs4, k4, ALU.mult)

                e_t = e_pool.tile([128, Tc, C, C], F32)
                nc.scalar.activation(e_t[:], s_t[:], AF.Exp)

                z_t = z_pool.tile([128, Tc, C], F32, tag="z")
                nc.vector.tensor_reduce(z_t[:], e_t[:], AX.X, ALU.add)
                zi_t = z_pool.tile([128, Tc, C], F32, tag="zi")
                nc.vector.reciprocal(zi_t[:], z_t[:])
                w_t = z_pool.tile([128, Tc, C], F32, tag="w")
                ew.tensor_tensor(w_t[:], v_sb[:, tsl, :], zi_t[:], ALU.mult)

                # P = E * W (broadcast over e), written TRANSPOSED into s_t:
                # s_t now holds P as [p, tau, m, c] so the c-reduce is contiguous
                w4 = w_t[:].unsqueeze(3).broadcast_to([128, Tc, C, C])
                ew.tensor_tensor(s_t[:].transpose([0, 1, 3, 2]), e_t[:], w4, ALU.mult)

                # ctx[p, tau, m] = sum_c P[p, tau, m, c]
                ctx_t = ctx_pool.tile([128, Tc, C], F32)
                nc.vector.tensor_reduce(ctx_t[:], s_t[:], AX.X, ALU.add)

                # out-proj: out[tau, m, o] = sum_d ctx[(t2,d),(tau,m)] * Wout[d, o]
                # bout folded in via a K=1 accumulating matmul of ones x bout
                for t2 in range(T2):
                    dp = slice(64 * t2, 64 * t2 + 64)
                    po = ps2.tile([128, K], F32)
                    nc.tensor.matmul(
                        po[:],
                        ctx_t[dp, :, :].rearrange("p t c -> p (t c)"),
                        wo_sb[dp, :],
                        start=True,
                        stop=False,
                    )
                    nc.tensor.matmul(
                        po[:],
                        ones_row[0:1, :],
                        bo_sb[0:1, :],
                        start=False,
                        stop=True,
                    )
                    nc.scalar.copy(stage[t2][:, g, :], po[:])

                if g == flush - 1:
                    chb = ch // flush
                    ov = out_d[:].rearrange(
                        "(t2 chb chs tau) (m o) -> t2 chb tau m chs o",
                        t2=T2,
                        chb=NCH // flush,
                        chs=flush,
                        tau=Tc,
                        m=C,
                    )
                    for t2 in range(T2):
                        nc.sync.dma_start(ov[t2, chb], stage[t2][:])

    nc.compile()
    return nc


def _host_prep(x, Wqkv, bqkv, Wout, bout):
    x = np.ascontiguousarray(np.asarray(x, dtype=np.float32)).reshape(-1, F)
    Wqkv = np.asarray(Wqkv, dtype=np.float32)
    bqkv = np.asarray(bqkv, dtype=np.float32)
    Wout = np.asarray(Wout, dtype=np.float32)
    bout = np.asarray(bout, dtype=np.float32)
    scale = 1.0 / np.sqrt(K)

    common = {
        "wq": np.ascontiguousarray((Wqkv[:, :, :K] * scale).reshape(F, K)),
        "wk": np.ascontiguousarray(Wqkv[:, :, K : 2 * K].reshape(F, K)),
        "wv": np.ascontiguousarray(Wqkv[:, :, 2 * K :].reshape(F, K)),
        "bq": np.ascontiguousarray(np.tile((bqkv[:, :K] * scale).T, (2, 1))),
        "bk": np.ascontiguousarray(np.tile(bqkv[:, K : 2 * K].T, (2, 1))),
        "bv": np.ascontiguousarray(np.tile(bqkv[:, 2 * K :].T, (2, 1))),
        "wout": np.ascontiguousarray(np.tile(Wout, (2, 1))),
        "bout": np.ascontiguousarray(np.tile(bout[None, :], (128, 1))),
    }
    return x, common


_NC_CACHE = {}


def _get_nc(T):
    if T not in _NC_CACHE:
        _NC_CACHE[T] = build_nc(T=T)
    return _NC_CACHE[T]


def kernel(x, Wqkv, bqkv, Wout, bout, _trace=False):
    from concourse.bass_utils import run_bass_kernel_spmd

    xs, common = _host_prep(x, Wqkv, bqkv, Wout, bout)
    n_tok = xs.shape[0]
    tpc = n_tok // N_CORES
    in_maps = [
        {**common, "x": np.ascontiguousarray(xs[i * tpc : (i + 1) * tpc])}
        for i in range(N_CORES)
    ]
    nc = _get_nc(tpc)
    res = run_bass_kernel_spmd(nc, in_maps, list(range(N_CORES)), trace=_trace)
    out = np.concatenate([res.results[i]["out"] for i in range(N_CORES)], axis=0)
    out = out.reshape(np.asarray(x).shape)
    if _trace:
        kernel.last_results = res
    return out


# revision 12
# speedup vs baseline: 1.1904x; 1.1532x over previous
"""Trainium2 Bass kernel for nn_DepthwiseCrossViTMAE (criss-cross multihead self-attention).

Reference computation per token t (B*L = 4096 tokens, hidden 2048 = C*K with C=32, K=64):
  qkv[c, :] = x[t, c*64:(c+1)*64] @ Wqkv[c] + bqkv[c]          (per-channel linear)
  q, k, v = split(qkv)                                          each (C, K)
  for each d in [0, 64):  S_d = outer(q[:, d], k[:, d]) / 8     (C x C)
                          A_d = softmax_rows(S_d)
                          ctx[d, m] = sum_c A_d[c, m] * v[c, d]
  out[t, c*64:(c+1)*64] = ctx.T[c] @ Wout + bout

Sharding: data-parallel over the 4096 tokens, 512 tokens per core on 8 cores.

Per-core layout: SBUF partitions p = (t2, d) with t2 in {0,1}, d in [0,64);
token t = t2*256 + tau.  q/k/v live as [p, (tau, c)].  The scores tensor is
built per chunk of Tc tau's as [p, (tau, c, e)] by a broadcast tensor_tensor
outer product on DVE, exp on ACT, row-sums (over e) and weighted column-sums
(over c) as segmented tensor_reduce on DVE.  PE does the QKV projections
(contracting k=64 per channel), the x transposes, and the output projection.
"""

import sys

sys.path.insert(0, "/opt/trn_rl_repo")

import numpy as np
from contextlib import ExitStack

import concourse.bass as bass
import concourse.bacc as bacc
import concourse.mybir as mybir
import concourse.tile as tile
from concourse.masks import make_identity

F32 = mybir.dt.float32
AF = mybir.ActivationFunctionType
ALU = mybir.AluOpType
AX = mybir.AxisListType

C = 32          # channels
K = 64          # per-channel width (also Wout dim)
F = C * K       # hidden = 2048
N_CORES = 8


def build_nc(T=512, Tc=4, flush=8, gp_num=6, gp_den=8):
    """Build the single-core Bass module for T tokens.

    Tc: tau-chunk size for the attention loop (free size per DVE op = Tc*1024).
    flush: chunks per output-DMA flush group.
    gp_num/gp_den: chunks with ch % gp_den < gp_num run their elementwise
    score/P passes on GPSIMD instead of DVE (engine load balancing).
    """
    T2 = 2
    TH = T // T2            # tokens per half (tau range)
    NFC = F // 128          # 16 feature chunks of x
    NTT = (T + 127) // 128  # token tiles of x
    NCH = TH // Tc          # attention chunks
    assert TH % Tc == 0 and NCH % flush == 0

    nc = bacc.Bacc()
    x_d = nc.dram_tensor("x", [T, F], F32, kind="ExternalInput")
    wq_d = nc.dram_tensor("wq", [F, K], F32, kind="ExternalInput")
    wk_d = nc.dram_tensor("wk", [F, K], F32, kind="ExternalInput")
    wv_d = nc.dram_tensor("wv", [F, K], F32, kind="ExternalInput")
    bq_d = nc.dram_tensor("bq", [128, C], F32, kind="ExternalInput")
    bk_d = nc.dram_tensor("bk", [128, C], F32, kind="ExternalInput")
    bv_d = nc.dram_tensor("bv", [128, C], F32, kind="ExternalInput")
    wo_d = nc.dram_tensor("wout", [128, K], F32, kind="ExternalInput")
    bo_d = nc.dram_tensor("bout", [128, K], F32, kind="ExternalInput")
    out_d = nc.dram_tensor("out", [T, F], F32, kind="ExternalOutput")

    with ExitStack() as octx:
        tc = octx.enter_context(tile.TileContext(nc))
        const_pool = octx.enter_context(tc.tile_pool(name="const", bufs=1))
        qkv_pool = octx.enter_context(tc.tile_pool(name="qkv", bufs=1))

        ident = const_pool.tile([128, 128], F32)
        make_identity(nc, ident[:])
        ones_row = const_pool.tile([1, 128], F32)
        nc.gpsimd.memset(ones_row[:], 1.0)

        bq_sb = const_pool.tile([128, C], F32)
        bk_sb = const_pool.tile([128, C], F32)
        bv_sb = const_pool.tile([128, C], F32)
        wo_sb = const_pool.tile([128, K], F32)
        bo_sb = const_pool.tile([128, K], F32)
        nc.sync.dma_start(bq_sb[:], bq_d[:])
        nc.sync.dma_start(bk_sb[:], bk_d[:])
        nc.sync.dma_start(bv_sb[:], bv_d[:])
        nc.sync.dma_start(wo_sb[:], wo_d[:])
        nc.sync.dma_start(bo_sb[:], bo_d[:])

        # q is pre-scaled by 1/sqrt(K) on the host (weights and bias).
        qs_sb = qkv_pool.tile([128, TH, C], F32)
        k_sb = qkv_pool.tile([128, TH, C], F32)
        v_sb = qkv_pool.tile([128, TH, C], F32)

        # ---------------- phase 1: load x, transpose, QKV projections ----
        with (
            tc.tile_pool(name="xload", bufs=2) as xpool,
            tc.tile_pool(name="xt", bufs=1) as xtpool,
            tc.tile_pool(name="wgt", bufs=1) as wpool,
            tc.tile_pool(name="ps_qkv", bufs=6, space="PSUM") as ps1,
            tc.tile_pool(name="ps_tr", bufs=2, space="PSUM") as pst,
        ):
            wq_sb = wpool.tile([128, NFC, K], F32)
            wk_sb = wpool.tile([128, NFC, K], F32)
            wv_sb = wpool.tile([128, NFC, K], F32)
            nc.sync.dma_start(wq_sb[:], wq_d[:].rearrange("(fc p) d -> p fc d", p=128))
            nc.sync.dma_start(wk_sb[:], wk_d[:].rearrange("(fc p) d -> p fc d", p=128))
            nc.sync.dma_start(wv_sb[:], wv_d[:].rearrange("(fc p) d -> p fc d", p=128))

            # xT[feat, token] per 128-feature chunk, via PE transpose
            xt_sb = xtpool.tile([128, NFC, T], F32)
            for tt in range(NTT):
                trows = min(128, T - tt * 128)
                x_sb = xpool.tile([128, F], F32)
                for fc in range(NFC):
                    nc.sync.dma_start(
                        x_sb[:trows, fc * 128 : (fc + 1) * 128],
                        x_d[tt * 128 : tt * 128 + trows, fc * 128 : (fc + 1) * 128],
                    )
                for fc in range(NFC):
                    ps_t = pst.tile([128, 128], F32)
                    nc.tensor.transpose(
                        ps_t[:, :trows],
                        x_sb[:trows, fc * 128 : (fc + 1) * 128],
                        ident[:trows, :trows],
                    )
                    nc.scalar.copy(
                        xt_sb[:, fc, tt * 128 : tt * 128 + trows], ps_t[:, :trows]
                    )

            # per-channel QKV projections
            for c in range(C):
                fc, h = divmod(c, 2)
                hp = slice(64 * h, 64 * h + 64)
                for w_sb, b_sb, dst in (
                    (wq_sb, bq_sb, qs_sb),
                    (wk_sb, bk_sb, k_sb),
                    (wv_sb, bv_sb, v_sb),
                ):
                    ps = ps1.tile([128, TH], F32)
                    for t2 in range(T2):
                        nc.tensor.matmul(
                            ps[64 * t2 : 64 * t2 + 64, :],
                            w_sb[hp, fc, :],
                            xt_sb[hp, fc, t2 * TH : (t2 + 1) * TH],
                            start=True,
                            stop=True,
                        )
                    nc.scalar.activation(
                        dst[:, :, c], ps[:], AF.Identity, bias=b_sb[:, c : c + 1]
                    )

        # ---------------- phase 2: criss-cross attention + out-proj ------
        with (
            tc.tile_pool(name="s", bufs=3) as s_pool,
            tc.tile_pool(name="e", bufs=2) as e_pool,
            tc.tile_pool(name="zsm", bufs=3) as z_pool,
            tc.tile_pool(name="ctx", bufs=2) as ctx_pool,
            tc.tile_pool(name="stage", bufs=2) as stage_pool,
            tc.tile_pool(name="ps_o", bufs=4, space="PSUM") as ps2,
        ):
            stage = [None, None]
            for ch in range(NCH):
                g = ch % flush
                if g == 0:
                    stage = [
                        stage_pool.tile([128, flush, K], F32, tag="st0", name="st0"),
                        stage_pool.tile([128, flush, K], F32, tag="st1", name="st1"),
                    ]
                tsl = slice(ch * Tc, (ch + 1) * Tc)
                on_gp = (ch % gp_den) < gp_num
                ew = nc.gpsimd if on_gp else nc.vector
                qs4 = qs_sb[:, tsl, :].unsqueeze(3).broadcast_to([128, Tc, C, C])
                k4 = k_sb[:, tsl, :].unsqueeze(2).broadcast_to([128, Tc, C, C])
                s_t = s_pool.tile([128, Tc, C, C], F32)
                ew.tensor_tensor(s_t[:], qs4, k4, ALU.mult)

                e_t = e_pool.tile([128, Tc, C, C], F32)
                nc.scalar.activation(e_t[:], s_t[:], AF.Exp)

                z_t = z_pool.tile([128, Tc, C], F32, tag="z")
                nc.vector.tensor_reduce(z_t[:], e_t[:], AX.X, ALU.add)
                zi_t = z_pool.tile([128, Tc, C], F32, tag="zi")
                nc.vector.reciprocal(zi_t[:], z_t[:])
                w_t = z_pool.tile([128, Tc, C], F32, tag="w")
                ew.tensor_tensor(w_t[:], v_sb[:, tsl, :], zi_t[:], ALU.mult)

                w4 = w_t[:].unsqueeze(3).broadcast_to([128, Tc, C, C])
                ctx_t = ctx_pool.tile([128, Tc, C], F32)
                if on_gp:
                    # GPSIMD pays no stride penalty: write P transposed
                    # ([p, tau, m, c]) so the DVE c-reduce is contiguous.
                    ew.tensor_tensor(
                        s_t[:].transpose([0, 1, 3, 2]), e_t[:], w4, ALU.mult
                    )
                    nc.vector.tensor_reduce(ctx_t[:], s_t[:], AX.X, ALU.add)
                else:
                    # DVE pays 1.6x for any strided op: keep P contiguous
                    # ([p, tau, c, m]) and reduce over c with a contiguous
                    # pairwise tree (sum halves over the middle axis).
                    nc.vector.tensor_tensor(s_t[:], e_t[:], w4, ALU.mult)
                    src = s_t[:]
                    width = C
                    while width > 1:
                        half = width // 2
                        lo = src[:, :, 0:half, :]
                        hi = src[:, :, half:width, :]
                        if half == 1:
                            nc.vector.tensor_tensor(
                                ctx_t[:], lo.squeeze(2), hi.squeeze(2), ALU.add
                            )
                        else:
                            nc.vector.tensor_tensor(lo, lo, hi, ALU.add)
                        width = half

                # out-proj: out[tau, m, o] = sum_d ctx[(t2,d),(tau,m)] * Wout[d, o]
                # bout folded in via a K=1 accumulating matmul of ones x bout
                for t2 in range(T2):
                    dp = slice(64 * t2, 64 * t2 + 64)
                    po = ps2.tile([128, K], F32)
                    nc.tensor.matmul(
                        po[:],
                        ctx_t[dp, :, :].rearrange("p t c -> p (t c)"),
                        wo_sb[dp, :],
                        start=True,
                        stop=False,
                    )
                    nc.tensor.matmul(
                        po[:],
                        ones_row[0:1, :],
                        bo_sb[0:1, :],
                        start=False,
                        stop=True,
                    )
                    nc.scalar.copy(stage[t2][:, g, :], po[:])

                if g == flush - 1:
                    chb = ch // flush
                    ov = out_d[:].rearrange(
                        "(t2 chb chs tau) (m o) -> t2 chb tau m chs o",
                        t2=T2,
                        chb=NCH // flush,
                        chs=flush,
                        tau=Tc,
                        m=C,
                    )
                    for t2 in range(T2):
                        nc.sync.dma_start(ov[t2, chb], stage[t2][:])

    nc.compile()
    return nc


def _host_prep(x, Wqkv, bqkv, Wout, bout):
    x = np.ascontiguousarray(np.asarray(x, dtype=np.float32)).reshape(-1, F)
    Wqkv = np.asarray(Wqkv, dtype=np.float32)
    bqkv = np.asarray(bqkv, dtype=np.float32)
    Wout = np.asarray(Wout, dtype=np.float32)
    bout = np.asarray(bout, dtype=np.float32)
    scale = 1.0 / np.sqrt(K)

    common = {
        "wq": np.ascontiguousarray((Wqkv[:, :, :K] * scale).reshape(F, K)),
        "wk": np.ascontiguousarray(Wqkv[:, :, K : 2 * K].reshape(F, K)),
        "wv": np.ascontiguousarray(Wqkv[:, :, 2 * K :].reshape(F, K)),
        "bq": np.ascontiguousarray(np.tile((bqkv[:, :K] * scale).T, (2, 1))),
        "bk": np.ascontiguousarray(np.tile(bqkv[:, K : 2 * K].T, (2, 1))),
        "bv": np.ascontiguousarray(np.tile(bqkv[:, 2 * K :].T, (2, 1))),
        "wout": np.ascontiguousarray(np.tile(Wout, (2, 1))),
        "bout": np.ascontiguousarray(np.tile(bout[None, :], (128, 1))),
    }
    return x, common


_NC_CACHE = {}


def _get_nc(T):
    if T not in _NC_CACHE:
        _NC_CACHE[T] = build_nc(T=T)
    return _NC_CACHE[T]


def kernel(x, Wqkv, bqkv, Wout, bout, _trace=False):
    from concourse.bass_utils import run_bass_kernel_spmd

    xs, common = _host_prep(x, Wqkv, bqkv, Wout, bout)
    n_tok = xs.shape[0]
    tpc = n_tok // N_CORES
    in_maps = [
        {**common, "x": np.ascontiguousarray(xs[i * tpc : (i + 1) * tpc])}
        for i in range(N_CORES)
    ]
    nc = _get_nc(tpc)
    res = run_bass_kernel_spmd(nc, in_maps, list(range(N_CORES)), trace=_trace)
    out = np.concatenate([res.results[i]["out"] for i in range(N_CORES)], axis=0)
    out = out.reshape(np.asarray(x).shape)
    if _trace:
        kernel.last_results = res
    return out


# revision 17
# speedup vs baseline: 1.2375x; 1.0396x over previous
"""Trainium2 Bass kernel for nn_DepthwiseCrossViTMAE (criss-cross multihead self-attention).

Reference computation per token t (B*L = 4096 tokens, hidden 2048 = C*K with C=32, K=64):
  qkv[c, :] = x[t, c*64:(c+1)*64] @ Wqkv[c] + bqkv[c]          (per-channel linear)
  q, k, v = split(qkv)                                          each (C, K)
  for each d in [0, 64):  S_d = outer(q[:, d], k[:, d]) / 8     (C x C)
                          A_d = softmax_rows(S_d)
                          ctx[d, m] = sum_c A_d[c, m] * v[c, d]
  out[t, c*64:(c+1)*64] = ctx.T[c] @ Wout + bout

Sharding: data-parallel over the 4096 tokens, 512 tokens per core on 8 cores.

Per-core layout: SBUF partitions p = (t2, d) with t2 in {0,1}, d in [0,64);
token t = t2*256 + tau.  q/k/v live as [p, (tau, c)].  The scores tensor is
built per chunk of Tc tau's as [p, (tau, c, e)] by a broadcast tensor_tensor
outer product on DVE, exp on ACT, row-sums (over e) and weighted column-sums
(over c) as segmented tensor_reduce on DVE.  PE does the QKV projections
(contracting k=64 per channel), the x transposes, and the output projection.
"""

import sys

sys.path.insert(0, "/opt/trn_rl_repo")

import numpy as np
from contextlib import ExitStack

import concourse.bass as bass
import concourse.bacc as bacc
import concourse.mybir as mybir
import concourse.tile as tile
from concourse.masks import make_identity

F32 = mybir.dt.float32
AF = mybir.ActivationFunctionType
ALU = mybir.AluOpType
AX = mybir.AxisListType

C = 32          # channels
K = 64          # per-channel width (also Wout dim)
F = C * K       # hidden = 2048
N_CORES = 8


def build_nc(T=512, Tc=2, flush=8, gp_num=13, gp_den=16):
    """Build the single-core Bass module for T tokens.

    Tc: tau-chunk size for the attention loop (free size per DVE op = Tc*1024).
    flush: chunks per output-DMA flush group.
    gp_num/gp_den: chunks with ch % gp_den < gp_num run their elementwise
    score/P passes on GPSIMD instead of DVE (engine load balancing).
    """
    T2 = 2
    TH = T // T2            # tokens per half (tau range)
    NFC = F // 128          # 16 feature chunks of x
    NTT = (T + 127) // 128  # token tiles of x
    NCH = TH // Tc          # attention chunks
    assert TH % Tc == 0 and NCH % flush == 0

    nc = bacc.Bacc()
    x_d = nc.dram_tensor("x", [T, F], F32, kind="ExternalInput")
    wq_d = nc.dram_tensor("wq", [F, K], F32, kind="ExternalInput")
    wk_d = nc.dram_tensor("wk", [F, K], F32, kind="ExternalInput")
    wv_d = nc.dram_tensor("wv", [F, K], F32, kind="ExternalInput")
    bq_d = nc.dram_tensor("bq", [128, C], F32, kind="ExternalInput")
    bk_d = nc.dram_tensor("bk", [128, C], F32, kind="ExternalInput")
    bv_d = nc.dram_tensor("bv", [128, C], F32, kind="ExternalInput")
    wo_d = nc.dram_tensor("wout", [128, K], F32, kind="ExternalInput")
    bo_d = nc.dram_tensor("bout", [128, K], F32, kind="ExternalInput")
    out_d = nc.dram_tensor("out", [T, F], F32, kind="ExternalOutput")

    with ExitStack() as octx:
        tc = octx.enter_context(tile.TileContext(nc))
        const_pool = octx.enter_context(tc.tile_pool(name="const", bufs=1))
        qkv_pool = octx.enter_context(tc.tile_pool(name="qkv", bufs=1))

        ident = const_pool.tile([128, 128], F32)
        make_identity(nc, ident[:])
        ones_row = const_pool.tile([1, 128], F32)
        nc.gpsimd.memset(ones_row[:], 1.0)

        bq_sb = const_pool.tile([128, C], F32)
        bk_sb = const_pool.tile([128, C], F32)
        bv_sb = const_pool.tile([128, C], F32)
        wo_sb = const_pool.tile([128, K], F32)
        bo_sb = const_pool.tile([128, K], F32)
        nc.sync.dma_start(bq_sb[:], bq_d[:])
        nc.sync.dma_start(bk_sb[:], bk_d[:])
        nc.sync.dma_start(bv_sb[:], bv_d[:])
        nc.sync.dma_start(wo_sb[:], wo_d[:])
        nc.sync.dma_start(bo_sb[:], bo_d[:])

        # q is pre-scaled by 1/sqrt(K) on the host (weights and bias).
        qs_sb = qkv_pool.tile([128, TH, C], F32)
        k_sb = qkv_pool.tile([128, TH, C], F32)
        v_sb = qkv_pool.tile([128, TH, C], F32)

        # ---------------- phase 1: load x, transpose, QKV projections ----
        with (
            tc.tile_pool(name="xload", bufs=2) as xpool,
            tc.tile_pool(name="xt", bufs=1) as xtpool,
            tc.tile_pool(name="wgt", bufs=1) as wpool,
            tc.tile_pool(name="ps_qkv", bufs=6, space="PSUM") as ps1,
            tc.tile_pool(name="ps_tr", bufs=2, space="PSUM") as pst,
        ):
            wq_sb = wpool.tile([128, NFC, K], F32)
            wk_sb = wpool.tile([128, NFC, K], F32)
            wv_sb = wpool.tile([128, NFC, K], F32)
            nc.sync.dma_start(wq_sb[:], wq_d[:].rearrange("(fc p) d -> p fc d", p=128))
            nc.sync.dma_start(wk_sb[:], wk_d[:].rearrange("(fc p) d -> p fc d", p=128))
            nc.sync.dma_start(wv_sb[:], wv_d[:].rearrange("(fc p) d -> p fc d", p=128))

            # xT[feat, token] per 128-feature chunk, via PE transpose
            xt_sb = xtpool.tile([128, NFC, T], F32)
            for tt in range(NTT):
                trows = min(128, T - tt * 128)
                x_sb = xpool.tile([128, F], F32)
                for fc in range(NFC):
                    nc.sync.dma_start(
                        x_sb[:trows, fc * 128 : (fc + 1) * 128],
                        x_d[tt * 128 : tt * 128 + trows, fc * 128 : (fc + 1) * 128],
                    )
                for fc in range(NFC):
                    ps_t = pst.tile([128, 128], F32)
                    nc.tensor.transpose(
                        ps_t[:, :trows],
                        x_sb[:trows, fc * 128 : (fc + 1) * 128],
                        ident[:trows, :trows],
                    )
                    nc.scalar.copy(
                        xt_sb[:, fc, tt * 128 : tt * 128 + trows], ps_t[:, :trows]
                    )

            # per-channel QKV projections, split into tau-halves so the
            # attention loop can start after the first half is done
            HH = TH // 2
            for half in range(2):
                hsl = slice(half * HH, half * HH + HH)
                for c in range(C):
                    fc, h = divmod(c, 2)
                    hp = slice(64 * h, 64 * h + 64)
                    for w_sb, b_sb, dst in (
                        (wq_sb, bq_sb, qs_sb),
                        (wk_sb, bk_sb, k_sb),
                        (wv_sb, bv_sb, v_sb),
                    ):
                        ps = ps1.tile([128, HH], F32)
                        for t2 in range(T2):
                            nc.tensor.matmul(
                                ps[64 * t2 : 64 * t2 + 64, :],
                                w_sb[hp, fc, :],
                                xt_sb[
                                    hp, fc, t2 * TH + half * HH : t2 * TH + half * HH + HH
                                ],
                                start=True,
                                stop=True,
                            )
                        nc.scalar.activation(
                            dst[:, hsl, c], ps[:], AF.Identity, bias=b_sb[:, c : c + 1]
                        )

        # ---------------- phase 2: criss-cross attention + out-proj ------
        with (
            tc.tile_pool(name="s", bufs=6) as s_pool,
            tc.tile_pool(name="e", bufs=4) as e_pool,
            tc.tile_pool(name="zsm", bufs=6) as z_pool,
            tc.tile_pool(name="ctx", bufs=4) as ctx_pool,
            tc.tile_pool(name="stage", bufs=2) as stage_pool,
            tc.tile_pool(name="ps_o", bufs=6, space="PSUM") as ps2,
        ):
            stage = [None, None]
            for ch in range(NCH):
                g = ch % flush
                if g == 0:
                    stage = [
                        stage_pool.tile(
                            [Tc * C, flush, K], F32, tag="st0", name="st0"
                        ),
                        stage_pool.tile(
                            [Tc * C, flush, K], F32, tag="st1", name="st1"
                        ),
                    ]
                tsl = slice(ch * Tc, (ch + 1) * Tc)
                on_gp = (ch % gp_den) < gp_num
                ew = nc.gpsimd if on_gp else nc.vector
                qs4 = qs_sb[:, tsl, :].unsqueeze(3).broadcast_to([128, Tc, C, C])
                k4 = k_sb[:, tsl, :].unsqueeze(2).broadcast_to([128, Tc, C, C])
                s_t = s_pool.tile([128, Tc, C, C], F32)
                ew.tensor_tensor(s_t[:], qs4, k4, ALU.mult)

                e_t = e_pool.tile([128, Tc, C, C], F32)
                nc.scalar.activation(e_t[:], s_t[:], AF.Exp)

                z_t = z_pool.tile([128, Tc, C], F32, tag="z")
                nc.vector.tensor_reduce(z_t[:], e_t[:], AX.X, ALU.add)
                zi_t = z_pool.tile([128, Tc, C], F32, tag="zi")
                nc.vector.reciprocal(zi_t[:], z_t[:])
                w_t = z_pool.tile([128, Tc, C], F32, tag="w")
                ew.tensor_tensor(w_t[:], v_sb[:, tsl, :], zi_t[:], ALU.mult)

                w4 = w_t[:].unsqueeze(3).broadcast_to([128, Tc, C, C])
                ctx_t = ctx_pool.tile([128, Tc, C], F32)
                if on_gp:
                    # GPSIMD pays no stride penalty: write P transposed
                    # ([p, tau, m, c]) so the DVE c-reduce is contiguous.
                    ew.tensor_tensor(
                        s_t[:].transpose([0, 1, 3, 2]), e_t[:], w4, ALU.mult
                    )
                    nc.vector.tensor_reduce(ctx_t[:], s_t[:], AX.X, ALU.add)
                else:
                    # DVE pays 1.6x for any strided op: keep P contiguous
                    # ([p, tau, c, m]) and reduce over c with a contiguous
                    # pairwise tree (sum halves over the middle axis).
                    nc.vector.tensor_tensor(s_t[:], e_t[:], w4, ALU.mult)
                    src = s_t[:]
                    width = C
                    while width > 1:
                        half = width // 2
                        lo = src[:, :, 0:half, :]
                        hi = src[:, :, half:width, :]
                        if half == 1:
                            nc.vector.tensor_tensor(
                                ctx_t[:], lo.squeeze(2), hi.squeeze(2), ALU.add
                            )
                        else:
                            nc.vector.tensor_tensor(lo, lo, hi, ALU.add)
                        width = half

                # out-proj: out[tau, m, o] = sum_d ctx[(t2,d),(tau,m)] * Wout[d, o]
                # bout folded in via a K=1 accumulating matmul of ones x bout
                for t2 in range(T2):
                    dp = slice(64 * t2, 64 * t2 + 64)
                    po = ps2.tile([Tc * C, K], F32)
                    nc.tensor.matmul(
                        po[:],
                        ctx_t[dp, :, :].rearrange("p t c -> p (t c)"),
                        wo_sb[dp, :],
                        start=True,
                        stop=False,
                    )
                    nc.tensor.matmul(
                        po[:],
                        ones_row[0:1, 0 : Tc * C],
                        bo_sb[0:1, :],
                        start=False,
                        stop=True,
                    )
                    nc.scalar.copy(stage[t2][:, g, :], po[:])

                if g == flush - 1:
                    chb = ch // flush
                    ov = out_d[:].rearrange(
                        "(t2 chb chs tau) (m o) -> t2 chb tau m chs o",
                        t2=T2,
                        chb=NCH // flush,
                        chs=flush,
                        tau=Tc,
                        m=C,
                    )
                    for t2 in range(T2):
                        nc.sync.dma_start(ov[t2, chb], stage[t2][:])

    nc.compile()
    return nc


def _host_prep(x, Wqkv, bqkv, Wout, bout):
    x = np.ascontiguousarray(np.asarray(x, dtype=np.float32)).reshape(-1, F)
    Wqkv = np.asarray(Wqkv, dtype=np.float32)
    bqkv = np.asarray(bqkv, dtype=np.float32)
    Wout = np.asarray(Wout, dtype=np.float32)
    bout = np.asarray(bout, dtype=np.float32)
    scale = 1.0 / np.sqrt(K)

    common = {
        "wq": np.ascontiguousarray((Wqkv[:, :, :K] * scale).reshape(F, K)),
        "wk": np.ascontiguousarray(Wqkv[:, :, K : 2 * K].reshape(F, K)),
        "wv": np.ascontiguousarray(Wqkv[:, :, 2 * K :].reshape(F, K)),
        "bq": np.ascontiguousarray(np.tile((bqkv[:, :K] * scale).T, (2, 1))),
        "bk": np.ascontiguousarray(np.tile(bqkv[:, K : 2 * K].T, (2, 1))),
        "bv": np.ascontiguousarray(np.tile(bqkv[:, 2 * K :].T, (2, 1))),
        "wout": np.ascontiguousarray(np.tile(Wout, (2, 1))),
        "bout": np.ascontiguousarray(np.tile(bout[None, :], (128, 1))),
    }
    return x, common


_NC_CACHE = {}


def _get_nc(T):
    if T not in _NC_CACHE:
        _NC_CACHE[T] = build_nc(T=T)
    return _NC_CACHE[T]


def kernel(x, Wqkv, bqkv, Wout, bout, _trace=False):
    from concourse.bass_utils import run_bass_kernel_spmd

    xs, common = _host_prep(x, Wqkv, bqkv, Wout, bout)
    n_tok = xs.shape[0]
    tpc = n_tok // N_CORES
    in_maps = [
        {**common, "x": np.ascontiguousarray(xs[i * tpc : (i + 1) * tpc])}
        for i in range(N_CORES)
    ]
    nc = _get_nc(tpc)
    res = run_bass_kernel_spmd(nc, in_maps, list(range(N_CORES)), trace=_trace)
    out = np.concatenate([res.results[i]["out"] for i in range(N_CORES)], axis=0)
    out = out.reshape(np.asarray(x).shape)
    if _trace:
        kernel.last_results = res
    return out


# revision 18
# speedup vs baseline: 1.2681x; 1.0247x over previous
"""Trainium2 Bass kernel for nn_DepthwiseCrossViTMAE (criss-cross multihead self-attention).

Reference computation per token t (B*L = 4096 tokens, hidden 2048 = C*K with C=32, K=64):
  qkv[c, :] = x[t, c*64:(c+1)*64] @ Wqkv[c] + bqkv[c]          (per-channel linear)
  q, k, v = split(qkv)                                          each (C, K)
  for each d in [0, 64):  S_d = outer(q[:, d], k[:, d]) / 8     (C x C)
                          A_d = softmax_rows(S_d)
                          ctx[d, m] = sum_c A_d[c, m] * v[c, d]
  out[t, c*64:(c+1)*64] = ctx.T[c] @ Wout + bout

Sharding: data-parallel over the 4096 tokens, 512 tokens per core on 8 cores.

Per-core layout: SBUF partitions p = (t2, d) with t2 in {0,1}, d in [0,64);
token t = t2*256 + tau.  q/k/v live as [p, (tau, c)].  The scores tensor is
built per chunk of Tc tau's as [p, (tau, c, e)] by a broadcast tensor_tensor
outer product on DVE, exp on ACT, row-sums (over e) and weighted column-sums
(over c) as segmented tensor_reduce on DVE.  PE does the QKV projections
(contracting k=64 per channel), the x transposes, and the output projection.
"""

import sys

sys.path.insert(0, "/opt/trn_rl_repo")

import numpy as np
from contextlib import ExitStack

import concourse.bass as bass
import concourse.bacc as bacc
import concourse.mybir as mybir
import concourse.tile as tile
from concourse.masks import make_identity

F32 = mybir.dt.float32
AF = mybir.ActivationFunctionType
ALU = mybir.AluOpType
AX = mybir.AxisListType

C = 32          # channels
K = 64          # per-channel width (also Wout dim)
F = C * K       # hidden = 2048
N_CORES = 8


def build_nc(T=512, Tc=2, flush=8, gp_num=13, gp_den=16):
    """Build the single-core Bass module for T tokens.

    Tc: tau-chunk size for the attention loop (free size per DVE op = Tc*1024).
    flush: chunks per output-DMA flush group.
    gp_num/gp_den: chunks with ch % gp_den < gp_num run their elementwise
    score/P passes on GPSIMD instead of DVE (engine load balancing).
    """
    T2 = 2
    TH = T // T2            # tokens per half (tau range)
    NFC = F // 128          # 16 feature chunks of x
    NTT = (T + 127) // 128  # token tiles of x
    NCH = TH // Tc          # attention chunks
    assert TH % Tc == 0 and NCH % flush == 0

    nc = bacc.Bacc()
    x_d = nc.dram_tensor("x", [T, F], F32, kind="ExternalInput")
    wq_d = nc.dram_tensor("wq", [F, K], F32, kind="ExternalInput")
    wk_d = nc.dram_tensor("wk", [F, K], F32, kind="ExternalInput")
    wv_d = nc.dram_tensor("wv", [F, K], F32, kind="ExternalInput")
    bq_d = nc.dram_tensor("bq", [128, C], F32, kind="ExternalInput")
    bk_d = nc.dram_tensor("bk", [128, C], F32, kind="ExternalInput")
    bv_d = nc.dram_tensor("bv", [128, C], F32, kind="ExternalInput")
    wo_d = nc.dram_tensor("wout", [128, K], F32, kind="ExternalInput")
    bo_d = nc.dram_tensor("bout", [128, K], F32, kind="ExternalInput")
    out_d = nc.dram_tensor("out", [T, F], F32, kind="ExternalOutput")

    with ExitStack() as octx:
        tc = octx.enter_context(tile.TileContext(nc))
        const_pool = octx.enter_context(tc.tile_pool(name="const", bufs=1))
        qkv_pool = octx.enter_context(tc.tile_pool(name="qkv", bufs=1))

        ident = const_pool.tile([128, 128], F32)
        make_identity(nc, ident[:])
        ones_row = const_pool.tile([1, 128], F32)
        nc.gpsimd.memset(ones_row[:], 1.0)

        bq_sb = const_pool.tile([128, C], F32)
        bk_sb = const_pool.tile([128, C], F32)
        bv_sb = const_pool.tile([128, C], F32)
        wo_sb = const_pool.tile([128, K], F32)
        bo_sb = const_pool.tile([128, K], F32)
        nc.sync.dma_start(bq_sb[:], bq_d[:])
        nc.sync.dma_start(bk_sb[:], bk_d[:])
        nc.sync.dma_start(bv_sb[:], bv_d[:])
        nc.sync.dma_start(wo_sb[:], wo_d[:])
        nc.sync.dma_start(bo_sb[:], bo_d[:])

        # q is pre-scaled by 1/sqrt(K) on the host (weights and bias).
        qs_sb = qkv_pool.tile([128, TH, C], F32)
        k_sb = qkv_pool.tile([128, TH, C], F32)
        v_sb = qkv_pool.tile([128, TH, C], F32)

        # ---------------- phase 1: load x, transpose, QKV projections ----
        with (
            tc.tile_pool(name="xload", bufs=2) as xpool,
            tc.tile_pool(name="xt", bufs=1) as xtpool,
            tc.tile_pool(name="wgt", bufs=1) as wpool,
            tc.tile_pool(name="ps_qkv", bufs=6, space="PSUM") as ps1,
            tc.tile_pool(name="ps_tr", bufs=2, space="PSUM") as pst,
        ):
            wq_sb = wpool.tile([128, NFC, K], F32)
            wk_sb = wpool.tile([128, NFC, K], F32)
            wv_sb = wpool.tile([128, NFC, K], F32)
            nc.sync.dma_start(wq_sb[:], wq_d[:].rearrange("(fc p) d -> p fc d", p=128))
            nc.sync.dma_start(wk_sb[:], wk_d[:].rearrange("(fc p) d -> p fc d", p=128))
            nc.sync.dma_start(wv_sb[:], wv_d[:].rearrange("(fc p) d -> p fc d", p=128))

            # xT[feat, token] per 128-feature chunk, via PE transpose
            xt_sb = xtpool.tile([128, NFC, T], F32)
            for tt in range(NTT):
                trows = min(128, T - tt * 128)
                x_sb = xpool.tile([128, F], F32)
                for fc in range(NFC):
                    nc.sync.dma_start(
                        x_sb[:trows, fc * 128 : (fc + 1) * 128],
                        x_d[tt * 128 : tt * 128 + trows, fc * 128 : (fc + 1) * 128],
                    )
                for fc in range(NFC):
                    ps_t = pst.tile([128, 128], F32)
                    nc.tensor.transpose(
                        ps_t[:, :trows],
                        x_sb[:trows, fc * 128 : (fc + 1) * 128],
                        ident[:trows, :trows],
                    )
                    nc.scalar.copy(
                        xt_sb[:, fc, tt * 128 : tt * 128 + trows], ps_t[:, :trows]
                    )

            # per-channel QKV projections, split into tau-halves so the
            # attention loop can start after the first half is done
            HH = TH // 2
            for half in range(2):
                hsl = slice(half * HH, half * HH + HH)
                for c in range(C):
                    fc, h = divmod(c, 2)
                    hp = slice(64 * h, 64 * h + 64)
                    for w_sb, b_sb, dst in (
                        (wq_sb, bq_sb, qs_sb),
                        (wk_sb, bk_sb, k_sb),
                        (wv_sb, bv_sb, v_sb),
                    ):
                        ps = ps1.tile([128, HH], F32)
                        for t2 in range(T2):
                            nc.tensor.matmul(
                                ps[64 * t2 : 64 * t2 + 64, :],
                                w_sb[hp, fc, :],
                                xt_sb[
                                    hp, fc, t2 * TH + half * HH : t2 * TH + half * HH + HH
                                ],
                                start=True,
                                stop=True,
                            )
                        nc.scalar.activation(
                            dst[:, hsl, c], ps[:], AF.Identity, bias=b_sb[:, c : c + 1]
                        )

        # ---------------- phase 2: criss-cross attention + out-proj ------
        with (
            tc.tile_pool(name="s", bufs=6) as s_pool,
            tc.tile_pool(name="e", bufs=4) as e_pool,
            tc.tile_pool(name="zsm", bufs=6) as z_pool,
            tc.tile_pool(name="ctx", bufs=4) as ctx_pool,
            tc.tile_pool(name="stage", bufs=2) as stage_pool,
            tc.tile_pool(name="ps_o", bufs=6, space="PSUM") as ps2,
        ):
            stage = [None, None]
            for ch in range(NCH):
                g = ch % flush
                if g == 0:
                    stage = [
                        stage_pool.tile(
                            [Tc * C, flush, K], F32, tag="st0", name="st0"
                        ),
                        stage_pool.tile(
                            [Tc * C, flush, K], F32, tag="st1", name="st1"
                        ),
                    ]
                tsl = slice(ch * Tc, (ch + 1) * Tc)
                on_gp = (ch % gp_den) < gp_num
                ew = nc.gpsimd if on_gp else nc.vector
                qs4 = qs_sb[:, tsl, :].unsqueeze(3).broadcast_to([128, Tc, C, C])
                k4 = k_sb[:, tsl, :].unsqueeze(2).broadcast_to([128, Tc, C, C])
                s_t = s_pool.tile([128, Tc, C, C], F32)
                ew.tensor_tensor(s_t[:], qs4, k4, ALU.mult)

                e_t = e_pool.tile([128, Tc, C, C], F32)
                nc.scalar.activation(e_t[:], s_t[:], AF.Exp)

                z_t = z_pool.tile([128, Tc, C], F32, tag="z")
                nc.vector.tensor_reduce(z_t[:], e_t[:], AX.X, ALU.add)
                zi_t = z_pool.tile([128, Tc, C], F32, tag="zi")
                nc.vector.reciprocal_approx_fast(zi_t[:], z_t[:])
                w_t = z_pool.tile([128, Tc, C], F32, tag="w")
                ew.tensor_tensor(w_t[:], v_sb[:, tsl, :], zi_t[:], ALU.mult)

                w4 = w_t[:].unsqueeze(3).broadcast_to([128, Tc, C, C])
                ctx_t = ctx_pool.tile([128, Tc, C], F32)
                if on_gp:
                    # GPSIMD pays no stride penalty: write P transposed
                    # ([p, tau, m, c]) so the DVE c-reduce is contiguous.
                    ew.tensor_tensor(
                        s_t[:].transpose([0, 1, 3, 2]), e_t[:], w4, ALU.mult
                    )
                    nc.vector.tensor_reduce(ctx_t[:], s_t[:], AX.X, ALU.add)
                else:
                    # DVE pays 1.6x for any strided op: keep P contiguous
                    # ([p, tau, c, m]) and reduce over c with a contiguous
                    # pairwise tree (sum halves over the middle axis).
                    nc.vector.tensor_tensor(s_t[:], e_t[:], w4, ALU.mult)
                    src = s_t[:]
                    width = C
                    while width > 1:
                        half = width // 2
                        lo = src[:, :, 0:half, :]
                        hi = src[:, :, half:width, :]
                        if half == 1:
                            nc.vector.tensor_tensor(
                                ctx_t[:], lo.squeeze(2), hi.squeeze(2), ALU.add
                            )
                        else:
                            nc.vector.tensor_tensor(lo, lo, hi, ALU.add)
                        width = half

                # out-proj: out[tau, m, o] = sum_d ctx[(t2,d),(tau,m)] * Wout[d, o]
                # bout folded in via a K=1 accumulating matmul of ones x bout
                for t2 in range(T2):
                    dp = slice(64 * t2, 64 * t2 + 64)
                    po = ps2.tile([Tc * C, K], F32)
                    nc.tensor.matmul(
                        po[:],
                        ctx_t[dp, :, :].rearrange("p t c -> p (t c)"),
                        wo_sb[dp, :],
                        start=True,
                        stop=False,
                    )
                    nc.tensor.matmul(
                        po[:],
                        ones_row[0:1, 0 : Tc * C],
                        bo_sb[0:1, :],
                        start=False,
                        stop=True,
                    )
                    nc.scalar.copy(stage[t2][:, g, :], po[:])

                if g == flush - 1:
                    chb = ch // flush
                    ov = out_d[:].rearrange(
                        "(t2 chb chs tau) (m o) -> t2 chb tau m chs o",
                        t2=T2,
                        chb=NCH // flush,
                        chs=flush,
                        tau=Tc,
                        m=C,
                    )
                    for t2 in range(T2):
                        nc.sync.dma_start(ov[t2, chb], stage[t2][:])

    nc.compile()
    return nc


def _host_prep(x, Wqkv, bqkv, Wout, bout):
    x = np.ascontiguousarray(np.asarray(x, dtype=np.float32)).reshape(-1, F)
    Wqkv = np.asarray(Wqkv, dtype=np.float32)
    bqkv = np.asarray(bqkv, dtype=np.float32)
    Wout = np.asarray(Wout, dtype=np.float32)
    bout = np.asarray(bout, dtype=np.float32)
    scale = 1.0 / np.sqrt(K)

    common = {
        "wq": np.ascontiguousarray((Wqkv[:, :, :K] * scale).reshape(F, K)),
        "wk": np.ascontiguousarray(Wqkv[:, :, K : 2 * K].reshape(F, K)),
        "wv": np.ascontiguousarray(Wqkv[:, :, 2 * K :].reshape(F, K)),
        "bq": np.ascontiguousarray(np.tile((bqkv[:, :K] * scale).T, (2, 1))),
        "bk": np.ascontiguousarray(np.tile(bqkv[:, K : 2 * K].T, (2, 1))),
        "bv": np.ascontiguousarray(np.tile(bqkv[:, 2 * K :].T, (2, 1))),
        "wout": np.ascontiguousarray(np.tile(Wout, (2, 1))),
        "bout": np.ascontiguousarray(np.tile(bout[None, :], (128, 1))),
    }
    return x, common


_NC_CACHE = {}


def _get_nc(T):
    if T not in _NC_CACHE:
        _NC_CACHE[T] = build_nc(T=T)
    return _NC_CACHE[T]


def kernel(x, Wqkv, bqkv, Wout, bout, _trace=False):
    from concourse.bass_utils import run_bass_kernel_spmd

    xs, common = _host_prep(x, Wqkv, bqkv, Wout, bout)
    n_tok = xs.shape[0]
    tpc = n_tok // N_CORES
    in_maps = [
        {**common, "x": np.ascontiguousarray(xs[i * tpc : (i + 1) * tpc])}
        for i in range(N_CORES)
    ]
    nc = _get_nc(tpc)
    res = run_bass_kernel_spmd(nc, in_maps, list(range(N_CORES)), trace=_trace)
    out = np.concatenate([res.results[i]["out"] for i in range(N_CORES)], axis=0)
    out = out.reshape(np.asarray(x).shape)
    if _trace:
        kernel.last_results = res
    return out
